# revision 4
# baseline (speedup 1.0000x reference)
"""Distributed Trainium2 kernel for nn_AncProbsLayer.

Math (reference):
    tau[b,h]  = softplus(tau_kernel[h, rate_indices[b,h]])
    R,p,Q     from tiny (H,K,20,20) kernels; Sm = D^1/2 Q D^-1/2; lam,U = eigh(Sm)
    P[b,h,k]  = D^-1/2 U diag(exp(tau*lam)) U^T D^1/2
    out       = einsum('blhz,bhkzs->blhks', inputs, P)

Device algorithm (V,W tiny host-precomputed eigen matrices; E from a
device-side indirect-DMA gather of tau_kernel + softplus + exp):
    P_comb[b]  = BDV @ (diag(E[b]) @ BDW)          (40x80, per-batch stationary)
    out[b,l,:] = in[b,l,:] @ P_comb[b]             (one matmul per batch)

The reference inputs are ONE-HOT rows (jax.nn.one_hot of labels), which
this kernel verifies on the host and then exploits hard:
  - the streaming operand is exact in fp8 (values 0/1) -> 4.2 MB/core of
    input DMA instead of 16 MB (kernel checks one-hotness; a dense-input
    fallback variant streams bf16 instead),
  - a SINGLE bf16 stationary suffices (one-hot picks single P entries;
    error = bf16 rounding of P ~ 2^-9, far under the 2e-2 gate) -> 2
    matmuls per batch-pair instead of 6,
  - output is written as u8 fixed-point (P entries are probabilities in
    [0,1]): scale 255 is folded into the stationary and +0.5 rounding
    bias enters through an extra all-ones contract row (K=41), so PSUM
    holds 255*p+0.5 and the evacuation is a plain f32->u8 copy split
    between DVE and ACT at PSUM-bank granularity. Host dequantizes.
Distribution: data-parallel over batch B across 8 cores (128 b each);
tiny kernels + tau table replicated (no collectives). Even/odd batches
live on SBUF partitions 0-40 / 64-104 (disjoint PE row groups run
concurrently via tile_position).
"""

import numpy as np
import ml_dtypes

import concourse.bass as bass
import concourse.bacc as bacc
import concourse.mybir as mybir
from concourse.tile import TileContext
from concourse.masks import make_identity
from concourse.bass_utils import run_bass_kernel_spmd

# Problem constants (hardcoded per the harness contract)
B, L, H, K, S = 1024, 512, 2, 2, 20
NUM_RATES = 100000
NCORES = 8
BPC = B // NCORES          # 128 batches per core
ROWS = BPC * L             # 65536 stream rows per core
HZ = H * S                 # 40  (input feature dim)
HKS = H * K * S            # 80  (output feature dim)
CB = 32                    # batches per DMA chunk
F32 = mybir.dt.float32
BF16 = mybir.dt.bfloat16
U8 = mybir.dt.uint8
F8 = mybir.dt.float8e4
NPBF16 = np.dtype(ml_dtypes.bfloat16)
NPF8 = np.dtype(ml_dtypes.float8_e4m3)
ONE_F8_BITS = int(np.asarray(1.0, NPF8).view(np.uint8))

OUT_SCALE = 255.0          # u8 fixed-point scale (fast path)
DEQ_OFF = 0.25             # dequant offset: robust to trunc vs RTN casts

_NC_CACHE = {}
_LAST_MODE = ["onehot"]


def build_nc(reps=1, mode=None):
    """mode 'onehot': fp8 one-hot rhs, 255-scaled bf16 stationaries with a
    +0.5 bias row (K=41), u8 output.  mode 'dense': bf16 rhs, unscaled
    stationaries (K=40), bf16 output — correctness fallback for inputs
    that are not one-hot.  reps>1 repeats the main stream inside one NEFF
    (benchmarking only: (wall[R]-wall[1])/(R-1) cancels dispatch overhead).
    """
    if mode is None:
        mode = _LAST_MODE[0]
    onehot = mode == "onehot"
    KC = 41 if onehot else 40        # contract rows (incl. bias row)
    out_dt = U8 if onehot else BF16
    nc = bacc.Bacc(
        "TRN2", target_bir_lowering=False, debug=False, num_devices=NCORES
    )
    # input pre-packed on host as a 128-partition image: rows 0-39 = even-b
    # features (row 40 = ones bias), rows 64-103 = odd-b (104 = ones).
    # 128-partition DMAs keep all 16 SDMA ports balanced.
    in_dt = U8 if onehot else BF16
    in_img = nc.declare_dram_parameter("in_img", [128, ROWS // 2], in_dt, isOutput=False)
    tau_tab = nc.declare_dram_parameter("tau_tab", [H * NUM_RATES, 1], F32, isOutput=False)
    offs = nc.declare_dram_parameter("offs", [BPC, H], mybir.dt.int32, isOutput=False)
    bdvT = nc.declare_dram_parameter("bdvT", [HKS, HZ], F32, isOutput=False)
    bdw = nc.declare_dram_parameter("bdw", [HKS, HKS], F32, isOutput=False)
    lam_rep = nc.declare_dram_parameter("lam_rep", [BPC, HKS], F32, isOutput=False)
    out = nc.declare_dram_parameter("out", [HKS, ROWS], out_dt, isOutput=True)

    QB = 32                    # batches per pc quarter
    NQ = BPC // QB             # 4 quarters
    NCHUNK = BPC // CB         # 4 chunks
    CCOLS = (CB // 2) * L      # 8192 image cols per chunk
    NG = CB // 4               # 8 psum groups per chunk (4 batches each)
    with TileContext(nc) as tc:
        with (
            tc.tile_pool(name="const", bufs=1) as cpool,
            tc.tile_pool(name="setup", bufs=2) as spool,
            tc.tile_pool(name="inp", bufs=3) as ipool,
            tc.tile_pool(name="ost", bufs=3) as opool,
            tc.tile_pool(name="psO", bufs=2, space="PSUM") as psO,
        ):
            # ---- constants / setup ----
            bdvT_t = cpool.tile([HKS, HZ], dtype=F32)
            nc.sync.dma_start(out=bdvT_t[:], in_=bdvT[:])
            bdw_t = cpool.tile([HKS, HKS], dtype=F32)
            nc.sync.dma_start(out=bdw_t[:], in_=bdw[:])
            lam_t = cpool.tile([BPC, HKS], dtype=F32)
            nc.sync.dma_start(out=lam_t[:], in_=lam_rep[:])
            offs_t = cpool.tile([BPC, H], dtype=mybir.dt.int32)
            nc.sync.dma_start(out=offs_t[:], in_=offs[:])
            ident = cpool.tile([BPC, BPC], dtype=F32)
            make_identity(nc, ident[:])

            # ---- gather tau values: tau_raw[b,h] = tau_tab[offs[b,h]] ----
            tau_raw = cpool.tile([BPC, H], dtype=F32)
            for h in range(H):
                nc.gpsimd.indirect_dma_start(
                    out=tau_raw[:, h : h + 1],
                    out_offset=None,
                    in_=tau_tab[:],
                    in_offset=bass.IndirectOffsetOnAxis(
                        ap=offs_t[:, h : h + 1], axis=0
                    ),
                )
            # softplus(x) = ln(exp(x) + 1): the ACT table set
            # (natural_log_exp_and_others) has exp/ln/copy but no softplus.
            tau_ex = cpool.tile([BPC, H], dtype=F32)
            nc.scalar.activation(
                tau_ex[:], tau_raw[:], mybir.ActivationFunctionType.Exp
            )
            tau_sp = cpool.tile([BPC, H], dtype=F32)
            nc.scalar.activation(
                tau_sp[:], tau_ex[:], mybir.ActivationFunctionType.Ln, bias=1.0
            )

            # ---- E[b, hks] = exp(tau[b,h] * lam[hks]) ----
            E = cpool.tile([BPC, HKS], dtype=F32)
            for h in range(H):
                sl = slice(h * K * S, (h + 1) * K * S)
                nc.scalar.activation(
                    E[:, sl],
                    lam_t[:, sl],
                    mybir.ActivationFunctionType.Exp,
                    scale=tau_sp[:, h : h + 1],
                )
            # transpose E -> E_T [80, 128] so per-b columns are per-partition scalars
            e_psb = psO.tile([128, 2048], dtype=F32, space="PSUM", tag="o")
            nc.tensor.transpose(out=e_psb[:HKS, :BPC], in_=E[:], identity=ident[:])
            e_t = cpool.tile([HKS, BPC], dtype=F32)
            nc.vector.tensor_copy(out=e_t[:], in_=e_psb[:HKS, :BPC])

            # ---- setup phase: P_comb bf16 stationaries, in 4 PARITY
            # quarters (q = half*2 + b%2). Odd-parity quarters are produced
            # directly at partitions 64-103 via column tile_position on the
            # small matmuls. bdwe[:, i*80+j] = BDW[:, j] * E_T[:, b(i)] via
            # stride-0 broadcast APs; P_comb = BDV @ bdwe (fp32 matmuls).
            # bdvT is pre-scaled by OUT_SCALE on host in 'onehot' mode.
            e_t4 = e_t[:].rearrange("p (hh i two) -> p hh two i", two=2, i=QB)
            pc_q = []
            for q in range(NQ):
                hh, par = q // 2, q % 2
                bdwe = spool.tile([HKS, QB * HKS], dtype=F32, tag="bdwe")
                nc.gpsimd.tensor_mul(
                    bdwe[:].rearrange("p (b j) -> p b j", j=HKS),
                    bdw_t[:, None, :].to_broadcast([HKS, QB, HKS]),
                    e_t4[:, hh, par, :].to_broadcast([HKS, QB, HKS]),
                )
                pc_t = cpool.tile([128, QB * HKS], dtype=BF16, tag=f"pc{q}")
                pb = 64 * par           # partition base for this parity
                tp = (0, 64) if par else None
                if onehot:
                    # +0.5 rounding-bias row at partition pb+40: engine APs
                    # need 32-aligned base partitions, so memset the whole
                    # 41-row block (base pb) and let the P_comb copies below
                    # overwrite rows pb..pb+39.
                    nc.vector.memset(pc_t[pb : pb + HZ + 1, :], 0.5)
                for m in range((QB * HKS) // L):
                    cs = slice(m * L, (m + 1) * L)
                    pc_ps = psO.tile([128, 2048], dtype=F32, space="PSUM", tag="o")
                    nc.tensor.matmul(
                        pc_ps[pb : pb + HZ, 0:L],
                        lhsT=bdvT_t[:], rhs=bdwe[:, cs],
                        start=True, stop=True, tile_position=tp,
                    )
                    nc.scalar.copy(
                        out=pc_t[pb : pb + HZ, cs], in_=pc_ps[pb : pb + HZ, 0:L]
                    )
                pc_q.append(pc_t)

            # ---- main stream: 4 chunks x 32 batches; per 4 batches one
            # [128,2048] PSUM tile (4 banks) takes 4 matmuls (even/odd row
            # groups run concurrently), then DVE evacuates banks 0-1 and
            # ACT banks 2-3 (disjoint banks -> parallel PSUM reads).
            for _rep in range(reps):
              for ci in range(NCHUNK):
                  csl = slice(ci * CCOLS, (ci + 1) * CCOLS)
                  it = ipool.tile([128, CCOLS], dtype=in_dt, tag="it")
                  nc.sync.dma_start(out=it[:], in_=in_img[:, csl])
                  if onehot:
                      it_m = it[:].bitcast(F8)
                  else:
                      it_m = it[:]
                  ot = opool.tile([HKS, 2 * CCOLS], dtype=out_dt, tag="ot")
                  for g in range(NG):
                      o_ps = psO.tile([128, 2048], dtype=F32, space="PSUM", tag="o")
                      for u in range(2):        # two batch-pairs per group
                          be = ci * CB + g * 4 + 2 * u
                          bo = be + 1
                          qe = (be // 64) * 2 + (be % 2)
                          qo = (bo // 64) * 2 + (bo % 2)
                          pse = slice(((be % 64) // 2) * HKS, ((be % 64) // 2 + 1) * HKS)
                          pso = slice(((bo % 64) // 2) * HKS, ((bo % 64) // 2 + 1) * HKS)
                          xs = slice((g * 2 + u) * L, (g * 2 + u + 1) * L)
                          oe = slice(2 * u * L, (2 * u + 1) * L)
                          oo = slice((2 * u + 1) * L, (2 * u + 2) * L)
                          nc.tensor.matmul(
                              o_ps[0:HKS, oe], lhsT=pc_q[qe][0:KC, pse],
                              rhs=it_m[0:KC, xs], start=True, stop=True,
                          )
                          nc.tensor.matmul(
                              o_ps[0:HKS, oo], lhsT=pc_q[qo][64 : 64 + KC, pso],
                              rhs=it_m[64 : 64 + KC, xs], start=True, stop=True,
                          )
                      # evacuate: DVE takes PSUM banks 0-1, ACT banks 2-3
                      gofs = g * 2048
                      nc.vector.tensor_copy(
                          out=ot[:, gofs : gofs + 1024], in_=o_ps[0:HKS, 0:1024]
                      )
                      nc.scalar.copy(
                          out=ot[:, gofs + 1024 : gofs + 2048],
                          in_=o_ps[0:HKS, 1024:2048],
                      )
                  # out-DMA on the ACT HWDGE ring: keeps the SP ring free
                  # for input prefetch (no head-of-line blocking)
                  c0 = ci * 2 * CCOLS
                  nc.scalar.dma_start(out=out[:, c0 : c0 + 2 * CCOLS], in_=ot[:])
    nc.finalize()
    return nc


def _host_prep(exchangeability_kernel, equilibrium_kernel):
    """Tiny (H,K,20,20) eigen prep in float64 on host -> BDV, BDW, lam."""
    ek = exchangeability_kernel.astype(np.float64)
    eq = equilibrium_kernel.astype(np.float64)
    Rm = 0.5 * (ek + np.swapaxes(ek, -1, -2))
    Rm = np.logaddexp(0.0, Rm)  # softplus
    Rm = Rm * (1.0 - np.eye(S))
    # softmax
    em = eq - eq.max(axis=-1, keepdims=True)
    p = np.exp(em)
    p /= p.sum(axis=-1, keepdims=True)
    Q = Rm * p[..., None, :]
    row = Q.sum(axis=-1)
    Q = Q - row[..., :, None] * np.eye(S)
    mue = (p * row).sum(axis=-1)[..., None, None]
    Q = Q / np.maximum(mue, 1e-16)
    sqrt_p = np.sqrt(p)
    inv_sqrt_p = 1.0 / sqrt_p
    Sm = sqrt_p[..., :, None] * Q * inv_sqrt_p[..., None, :]
    Sm = 0.5 * (Sm + np.swapaxes(Sm, -1, -2))
    lam, U = np.linalg.eigh(Sm)  # (H,K,S), (H,K,S,S)

    BDV = np.zeros((HZ, HKS), dtype=np.float64)
    BDW = np.zeros((HKS, HKS), dtype=np.float64)
    for h in range(H):
        for k in range(K):
            c = h * K * S + k * S
            # V[z,s] = U[z,s]/sqrt(p[z]) ; rows = (h,z), cols = (h,k,s)
            BDV[h * S : (h + 1) * S, c : c + S] = inv_sqrt_p[h, k][:, None] * U[h, k]
            # BDW[(h,k,s),(h,k,j)] = sqrt(p[j]) * U[j,s]
            BDW[c : c + S, c : c + S] = (sqrt_p[h, k][:, None] * U[h, k]).T
    lam_flat = lam.reshape(HKS)
    return BDV.astype(np.float32), BDW.astype(np.float32), lam_flat.astype(np.float32)


def _is_onehot(inputs):
    if inputs.min() < 0.0:
        return False
    s1 = np.einsum("blhz->blh", inputs)
    s2 = np.einsum("blhz,blhz->blh", inputs, inputs)
    return bool(np.all(s1 == 1.0) and np.all(s2 == 1.0))


def kernel(inputs, rate_indices, tau_kernel, exchangeability_kernel, equilibrium_kernel):
    inputs = np.asarray(inputs, dtype=np.float32)
    rate_indices = np.asarray(rate_indices)
    tau_kernel = np.asarray(tau_kernel, dtype=np.float32)

    mode = "onehot" if _is_onehot(inputs) else "dense"
    _LAST_MODE[0] = mode
    onehot = mode == "onehot"

    BDV, BDW, lam_flat = _host_prep(
        np.asarray(exchangeability_kernel), np.asarray(equilibrium_kernel)
    )
    if onehot:
        BDV = BDV * OUT_SCALE
    BDV_T = np.ascontiguousarray(BDV.T)
    lam_rep = np.broadcast_to(lam_flat, (BPC, HKS)).copy()
    tau_tab = tau_kernel.reshape(H * NUM_RATES, 1)

    if ("nc", mode) not in _NC_CACHE:
        _NC_CACHE[("nc", mode)] = build_nc(mode=mode)
    nc = _NC_CACHE[("nc", mode)]
    _NC_CACHE["nc"] = nc

    in_maps = []
    for c in range(NCORES):
        bsl = slice(c * BPC, (c + 1) * BPC)
        # feature-major stream layout: [40, 65536]; even batches at
        # partitions 0-39, odd at 64-103, ones bias rows at 40/104
        inT_c = np.ascontiguousarray(inputs[bsl].reshape(BPC * L, HZ).T)
        if onehot:
            v3 = inT_c.astype(NPF8).view(np.uint8).reshape(HZ, BPC, L)
            img = np.zeros((128, ROWS // 2), dtype=np.uint8)
            img[HZ] = ONE_F8_BITS
            img[64 + HZ] = ONE_F8_BITS
        else:
            v3 = inT_c.astype(NPBF16).reshape(HZ, BPC, L)
            img = np.zeros((128, ROWS // 2), dtype=NPBF16)
        img[:HZ] = v3[:, 0::2].reshape(HZ, ROWS // 2)
        img[64 : 64 + HZ] = v3[:, 1::2].reshape(HZ, ROWS // 2)
        offs_c = (
            np.arange(H, dtype=np.int64)[None, :] * NUM_RATES
            + rate_indices[bsl].astype(np.int64)
        ).astype(np.int32)
        in_maps.append(
            {
                "in_img": img,
                "tau_tab": tau_tab,
                "offs": np.ascontiguousarray(offs_c),
                "bdvT": BDV_T,
                "bdw": BDW,
                "lam_rep": lam_rep,
            }
        )

    _NC_CACHE["in_maps"] = in_maps
    res = run_bass_kernel_spmd(nc, in_maps, core_ids=list(range(NCORES)))

    out = np.empty((B, L, H, K, S), dtype=np.float32)
    for c in range(NCORES):
        o = res.results[c]["out"]  # (80, 65536)
        if onehot:
            of = (o.astype(np.float32) - DEQ_OFF) * (1.0 / OUT_SCALE)
        else:
            of = o.astype(np.float32)
        out[c * BPC : (c + 1) * BPC] = of.T.reshape(BPC, L, H, K, S)
    return out
